# revision 16
# baseline (speedup 1.0000x reference)
"""DeepSeek-style fp8-quantized MLP (SwiGLU) on 8 Trainium2 NeuronCores.

Reference semantics (per reference.py):
    h  = fp8_gemm_nt(x, w_upgate)        # (1024, 4096)
    act = silu(h[:, :2048]) * h[:, 2048:]
    out = fp8_gemm_nt(act, w_down)       # (1024, 7168)
where fp8_gemm_nt quantizes the activation per-token/per-128-group and the
weight per-128x128-block to float8_e4m3fn (max 448), dequantizes, and
matmuls in fp32.

Distribution (tensor-parallel on the expert dim, 8 cores):
  - Weights are quantized+dequantized on the host exactly as the reference
    does (offline weight prep), cast to fp16, transposed to [k, n] layout and
    sharded: core i gets w_upgate columns for expert slice i (256 gate + 256
    up) and w_down output columns [896*i, 896*(i+1)).
  - x-quantization runs on device, sharded by rows: core i quantizes its own
    128-row slice of x (amax per 128-group, scale, fp8 round via the
    half-scale trick, dequantize to fp16), then an AllGather distributes the
    dequantized fp16 x to every core.
  - gemm1 accumulates over k in PSUM; SwiGLU and the act quantization run on
    the gemm1 output; act.T fp16 slices are AllGathered (in two halves so the
    second gather overlaps gemm2 of the first half); gemm2 writes this
    core's 896 output columns.

fp8 rounding on device uses the native float8e4 (TRN e4m3, max 240) cast at
HALF scale: the TRN e4m3 grid at v/2 coincides with the e4m3fn grid at v for
all |v| >= 2^-5 * 448-scale, so q*(2s) reproduces the reference dequantized
values exactly (validated numerically; residual differences only at the
subnormal boundary, ~1e-4 relative).
"""

import os
import sys

for _p in ("/opt/trn_rl_repo",):
    if _p not in sys.path:
        sys.path.insert(0, _p)

from contextlib import ExitStack

import ml_dtypes
import numpy as np

import concourse.bass as bass
import concourse.tile as tile
from concourse import bacc, mybir
from concourse.bass_utils import run_bass_kernel_spmd

# Problem shapes (hardcoded per the contract).
M, H, E = 1024, 7168, 2048
P = 128
NCORES = 8
KB = H // P          # 56 k-blocks for gemm1
EBLK = E // P        # 16 e-blocks for gemm2
ESH = E // NCORES    # 256 expert cols per core
HSH = H // NCORES    # 896 output cols per core
MT = M // P          # 8 m-tiles
FP8_MAX = 448.0

F32 = mybir.dt.float32
F16 = mybir.dt.float16
F8 = mybir.dt.float8e4

_PROGRAM_CACHE = {}
LAST_RESULTS = None


def _quant_dequant_wgt(w: np.ndarray) -> np.ndarray:
    """Exactly the reference's per-128x128-block weight quantize+dequantize."""
    n, k = w.shape
    wb = w.reshape(n // P, P, k // P, P)
    amax = np.max(np.abs(wb), axis=(1, 3), keepdims=True)
    scale = np.maximum(amax, np.float32(1e-4)) / np.float32(FP8_MAX)
    q = (wb / scale).astype(ml_dtypes.float8_e4m3fn)
    return (q.astype(np.float32) * scale).reshape(n, k)


def _build_program():
    nc = bacc.Bacc(
        "TRN2",
        target_bir_lowering=False,
        debug=False,
        num_devices=NCORES,
    )

    xs = nc.dram_tensor("xs", [P, H], F32, kind="ExternalInput")
    w1t = nc.dram_tensor("w1t", [H, 2 * ESH], F16, kind="ExternalInput")
    w2t = nc.dram_tensor("w2t", [E, HSH], F16, kind="ExternalInput")
    out = nc.dram_tensor("out", [M, HSH], F32, kind="ExternalOutput")

    groups = [list(range(NCORES))]
    AX = mybir.AxisListType.X
    MAX = mybir.AluOpType.max
    MULT = mybir.AluOpType.mult
    BYPASS = mybir.AluOpType.bypass

    with tile.TileContext(nc) as tc:
        with tc.tile_pool(name="dram", bufs=1, space="DRAM") as dram:
            dx_in = dram.tile([P, H], F16)
            dx_all = dram.tile([M, H], F16, addr_space="Shared")
            da_in0 = dram.tile([ESH, M // 2], F16)
            da_in1 = dram.tile([ESH, M // 2], F16)
            da_all0 = dram.tile([E, M // 2], F16, addr_space="Shared")
            da_all1 = dram.tile([E, M // 2], F16, addr_space="Shared")

            with ExitStack() as pools:
                wpool = pools.enter_context(tc.tile_pool(name="wpool", bufs=1))
                spool = pools.enter_context(tc.tile_pool(name="spool", bufs=1))

                # Resident weights: w1 [128, 56, 512] fp16, w2 [128, 16, 896].
                w1s = wpool.tile([P, KB, 2 * ESH], F16)
                nc.sync.dma_start(w1s, w1t.rearrange("(g p) n -> p g n", p=P))
                w2s = wpool.tile([P, EBLK, HSH], F16)
                nc.sync.dma_start(w2s, w2t.rearrange("(g p) n -> p g n", p=P))

                # ---- Phase Q: quantize this core's 128-row slice of x ----
                NCH = 8
                HC = H // NCH
                GH = HC // P  # 7 groups per chunk
                qpool = pools.enter_context(tc.tile_pool(name="qpool", bufs=2))
                dxh = qpool.tile([P, H], F16, bufs=1)
                for h in range(NCH):
                    xf = qpool.tile([P, HC], F32, tag="xf")
                    nc.sync.dma_start(xf, xs[:, h * HC:(h + 1) * HC])
                    xv = xf.rearrange("p (g b) -> p g b", b=P)
                    amax = qpool.tile([P, GH], F32, tag="amax")
                    nc.vector.tensor_reduce(
                        amax, xv, axis=AX, op=MAX, apply_absolute_value=True
                    )
                    # s2 = 2*scale = max(amax, 1e-4) * (2/448)
                    s2 = qpool.tile([P, GH], F32, tag="s2")
                    nc.vector.tensor_scalar(
                        s2, amax, 1e-4, float(2.0 / FP8_MAX), op0=MAX, op1=MULT
                    )
                    rcp = qpool.tile([P, GH], F32, tag="rcp")
                    nc.vector.reciprocal(rcp, s2)
                    q8 = qpool.tile([P, HC], F8, tag="q8")
                    nc.vector.tensor_tensor(
                        q8.rearrange("p (g b) -> p g b", b=P),
                        xv,
                        rcp.unsqueeze(2).broadcast_to([P, GH, P]),
                        op=MULT,
                    )
                    nc.vector.tensor_tensor(
                        dxh[:, h * HC:(h + 1) * HC].rearrange(
                            "p (g b) -> p g b", b=P
                        ),
                        q8.rearrange("p (g b) -> p g b", b=P),
                        s2.unsqueeze(2).broadcast_to([P, GH, P]),
                        op=MULT,
                    )
                nc.sync.dma_start(dx_in, dxh)

                nc.gpsimd.collective_compute(
                    "AllGather",
                    BYPASS,
                    replica_groups=groups,
                    ins=[dx_in.opt()],
                    outs=[dx_all.opt()],
                )

                # act.T accumulation tiles (fp16), one per m-half.
                daT0 = spool.tile([P, ESH // P, M // 2], F16)
                daT1 = spool.tile([P, ESH // P, M // 2], F16)

                EXP = mybir.ActivationFunctionType.Exp

                with (
                    tc.tile_pool(name="mmpool", bufs=2) as mmpool,
                    tc.tile_pool(name="apool", bufs=2) as apool,
                    tc.tile_pool(name="ps1", bufs=3, space="PSUM") as ps1pool,
                    tc.tile_pool(name="ps2", bufs=2, space="PSUM") as ps2pool,
                    tc.tile_pool(name="opool", bufs=2) as opool,
                    tc.tile_pool(name="atpool", bufs=1) as atpool,
                ):
                    actT0 = atpool.tile([P, EBLK, M // 2], F16)
                    actT1 = atpool.tile([P, EBLK, M // 2], F16)

                    # ---- gemm1 + SwiGLU + act quant, per m-tile ----
                    for j in range(MT):
                        dxT = mmpool.tile([P, KB, P], F16, tag="dxT")
                        nc.sync.dma_start(
                            dxT, dx_all[j * P:(j + 1) * P, :], transpose=True
                        )
                        ps = ps1pool.tile([P, 2 * ESH], F32, tag="g1")
                        for kb in range(KB):
                            nc.tensor.matmul(
                                ps,
                                lhsT=dxT[:, kb, :],
                                rhs=w1s[:, kb, :],
                                start=(kb == 0),
                                stop=(kb == KB - 1),
                            )
                        # SwiGLU: act = gate*up / (1 + exp(-gate))
                        eneg = apool.tile([P, ESH], F32, tag="eneg", bufs=1)
                        nc.scalar.activation(eneg, ps[:, 0:ESH], EXP, scale=-1.0)
                        den = apool.tile([P, ESH], F32, tag="den", bufs=1)
                        nc.vector.tensor_scalar_add(den, eneg, 1.0)
                        srec = apool.tile([P, ESH], F32, tag="srec", bufs=1)
                        nc.vector.reciprocal(srec, den)
                        ups = apool.tile([P, ESH], F32, tag="ups", bufs=1)
                        nc.scalar.copy(ups, ps[:, ESH:2 * ESH])
                        gu = apool.tile([P, ESH], F32, tag="gu", bufs=1)
                        nc.vector.tensor_mul(gu, ps[:, 0:ESH], ups)
                        av = apool.tile([P, ESH], F32, tag="av")
                        nc.vector.tensor_mul(av, gu, srec)
                        # act quantization (2 groups of 128)
                        GA = ESH // P
                        av3 = av.rearrange("p (g b) -> p g b", b=P)
                        am2 = apool.tile([P, GA], F32, tag="am2")
                        nc.vector.tensor_reduce(
                            am2, av3, axis=AX, op=MAX, apply_absolute_value=True
                        )
                        s2a = apool.tile([P, GA], F32, tag="s2a")
                        nc.vector.tensor_scalar(
                            s2a, am2, 1e-4, float(2.0 / FP8_MAX), op0=MAX, op1=MULT
                        )
                        r2 = apool.tile([P, GA], F32, tag="r2")
                        nc.vector.reciprocal(r2, s2a)
                        qa = apool.tile([P, ESH], F8, tag="qa")
                        nc.vector.tensor_tensor(
                            qa.rearrange("p (g b) -> p g b", b=P),
                            av3,
                            r2.unsqueeze(2).broadcast_to([P, GA, P]),
                            op=MULT,
                        )
                        dah = apool.tile([P, ESH], F16, tag="dah")
                        nc.vector.tensor_tensor(
                            dah.rearrange("p (g b) -> p g b", b=P),
                            qa.rearrange("p (g b) -> p g b", b=P),
                            s2a.unsqueeze(2).broadcast_to([P, GA, P]),
                            op=MULT,
                        )
                        dst = daT0 if j < 4 else daT1
                        c0 = (j % 4) * P
                        nc.sync.dma_start(
                            dst[:, :, c0:c0 + P], dah, transpose=True
                        )
                        if j == 3:
                            nc.sync.dma_start(
                                da_in0.rearrange("(g p) m -> p g m", p=P), daT0
                            )
                            nc.gpsimd.collective_compute(
                                "AllGather",
                                BYPASS,
                                replica_groups=groups,
                                ins=[da_in0.opt()],
                                outs=[da_all0.opt()],
                            )
                            nc.sync.dma_start(
                                actT0,
                                da_all0.rearrange("(g p) m -> p g m", p=P),
                            )
                        if j == MT - 1:
                            nc.sync.dma_start(
                                da_in1.rearrange("(g p) m -> p g m", p=P), daT1
                            )
                            nc.gpsimd.collective_compute(
                                "AllGather",
                                BYPASS,
                                replica_groups=groups,
                                ins=[da_in1.opt()],
                                outs=[da_all1.opt()],
                            )
                            nc.sync.dma_start(
                                actT1,
                                da_all1.rearrange("(g p) m -> p g m", p=P),
                            )

                    # ---- gemm2, per m-tile ----
                    NS0 = 512
                    for j in range(MT):
                        actT = actT0 if j < 4 else actT1
                        c0 = (j % 4) * P
                        psa = ps2pool.tile([P, NS0], F32, tag="g2a")
                        psb = ps2pool.tile([P, HSH - NS0], F32, tag="g2b")
                        for eb in range(EBLK):
                            nc.tensor.matmul(
                                psa,
                                lhsT=actT[:, eb, c0:c0 + P],
                                rhs=w2s[:, eb, 0:NS0],
                                start=(eb == 0),
                                stop=(eb == EBLK - 1),
                            )
                            nc.tensor.matmul(
                                psb,
                                lhsT=actT[:, eb, c0:c0 + P],
                                rhs=w2s[:, eb, NS0:HSH],
                                start=(eb == 0),
                                stop=(eb == EBLK - 1),
                            )
                        ot = opool.tile([P, HSH], F32, tag="ot")
                        nc.scalar.copy(ot[:, 0:NS0], psa)
                        nc.scalar.copy(ot[:, NS0:HSH], psb)
                        nc.sync.dma_start(out[j * P:(j + 1) * P, :], ot)

    nc.compile()
    return nc


def _get_program():
    if "nc" not in _PROGRAM_CACHE:
        _PROGRAM_CACHE["nc"] = _build_program()
    return _PROGRAM_CACHE["nc"]


def _prep_in_maps(x, w_upgate, w_down):
    x = np.ascontiguousarray(np.asarray(x, dtype=np.float32))
    w_upgate = np.asarray(w_upgate, dtype=np.float32)
    w_down = np.asarray(w_down, dtype=np.float32)

    dw1 = _quant_dequant_wgt(w_upgate)          # (4096, 7168)
    dw2 = _quant_dequant_wgt(w_down)            # (7168, 2048)

    dw1_f16 = dw1.astype(np.float16)
    dw2t_f16 = np.ascontiguousarray(dw2.T.astype(np.float16))  # (2048, 7168)

    in_maps = []
    for i in range(NCORES):
        gate = dw1_f16[i * ESH:(i + 1) * ESH, :]           # (256, 7168)
        up = dw1_f16[E + i * ESH:E + (i + 1) * ESH, :]     # (256, 7168)
        w1t_i = np.ascontiguousarray(
            np.concatenate([gate, up], axis=0).T           # (7168, 512)
        )
        w2t_i = np.ascontiguousarray(
            dw2t_f16[:, i * HSH:(i + 1) * HSH]             # (2048, 896)
        )
        in_maps.append(
            {
                "xs": np.ascontiguousarray(x[i * P:(i + 1) * P, :]),
                "w1t": w1t_i,
                "w2t": w2t_i,
            }
        )
    return in_maps


def _install_ntff_hook():
    """Register the axon NTFF profiling hook that bass_utils expects.

    The boot path degrades silently when the image's `antenv` package has no
    `axon_hooks` submodule; recreate it and wire up the ctypes-based hook so
    trace=True produces NTFF profiles."""
    import types

    try:
        from antenv.axon_hooks import get_axon_ntff_profile_hook  # noqa: F401

        return
    except ImportError:
        pass
    try:
        import antenv
    except ImportError:
        antenv = types.ModuleType("antenv")
        sys.modules["antenv"] = antenv
    mod = types.ModuleType("antenv.axon_hooks")
    _state = {"hook": None}
    mod.set_axon_ntff_profile_hook = lambda h: _state.__setitem__("hook", h)
    mod.get_axon_ntff_profile_hook = lambda: _state["hook"]
    sys.modules["antenv.axon_hooks"] = mod
    sys.modules["antenv"].axon_hooks = mod
    try:
        from trn_agent_boot.trn_boot import _ntff_profile_via_ctypes

        mod.set_axon_ntff_profile_hook(
            _ntff_profile_via_ctypes("/opt/axon/libaxon_pjrt.so")
        )
    except Exception:
        pass


def kernel(x, w_upgate, w_down):
    global LAST_RESULTS
    nc = _get_program()
    if bool(int(os.environ.get("KERNEL_TRACE", "0"))):
        _install_ntff_hook()
    in_maps = _prep_in_maps(x, w_upgate, w_down)
    res = run_bass_kernel_spmd(
        nc,
        in_maps,
        core_ids=list(range(NCORES)),
        trace=bool(int(os.environ.get("KERNEL_TRACE", "0"))),
    )
    LAST_RESULTS = res
    outs = [res.results[i]["out"] for i in range(NCORES)]
    return np.concatenate(outs, axis=1)
